# revision 41
# baseline (speedup 1.0000x reference)
"""Sharded KNN retrieval kernel for trn2 (8 NeuronCores).

Problem: nn_AnatomicalTextEnhancer — cosine-similarity top-k per (batch, region)
  query       [16, 29, 768] f32
  db_features [29, 8192, 768] f32
  exclude_idx [16, 29] i32
  top_k = 8
Returns (top_vals [16,29,8] f32, top_idx [16,29,8] i32, best_similarity [16,29] f32).

Sharding: db along N (1024 samples/core), query replicated; per-core local
top-8 then AllGather + re-top-8 of 64 on device (standard sharded KNN).

Per-core pipeline (fp32 exact; ~810-890 us HW exec):
  - db region tiles DMA'd natural [n, h] (3 MB strided loads, full HBM BW)
  - row norms^2 via Square+accum, split across ScalarE/GpSimd+DVE (knobs)
  - db transposed to [h, n] via PE transpose-mode matmuls (1-pass fp32),
    PSUM->SBUF copies split ScalarE/DVE
  - sims = qT.T @ dbT fp32 matmuls (M=16, N=512, PSUM-accumulated over 6
    h-chunks), software-pipelined one region behind the transposes so the
    PE stays dense
  - 1/||d|| folded into the PSUM->SBUF staging copy (free-axis multiply
    against a partition_broadcast row); 1/||q|| applied only to the 8
    final values (row-scaling cannot change ranking)
  - exclusion pre-masked via iota==local_excl -> -1e30 on packed
    [128, 1024] tiles (8 regions x 16 batch rows per pack)
  - local top-8 via DVE max/max_index; candidates (val, global idx as f32)
    AllGathered; final top-8-of-64 via max + exact value-match index
    recovery (min over idx-2^23 restricted to matches, ties -> lowest idx)
"""

import numpy as np

import concourse.bass as bass
import concourse.bacc as bacc
import concourse.mybir as mybir
from concourse.tile import TileContext
from concourse.bass_utils import run_bass_kernel_spmd

F32 = mybir.dt.float32
I32 = mybir.dt.int32
U16 = mybir.dt.uint16
U32 = mybir.dt.uint32

B, R, N, H = 16, 29, 8192, 768
NCORES = 8
NS = N // NCORES          # 1024 db samples per core
ROWS = B * R              # 464 (b-major: row = b*29 + r)
KC = H // 128             # 6 h-chunks
NBLK = NS // 128          # 8 n-blocks per region per core
NPACK = (R + 7) // 8      # 4 packs of 8 regions for 128-partition top-k tiles
NEG = -1e30
NORM_ACT = 4   # n-blocks per 8 whose norms run on ACT
NORM_GPS = 3   # n-blocks per 8 whose norm squares run on GPS (DVE reduces)
               # remaining blocks: DVE 1-pass via scalar_tensor_tensor accum
COPY_ACT = 5   # n-blocks per 8 whose PSUM->SBUF DT copy runs on ACT (rest DVE)
AF = mybir.ActivationFunctionType
ALU = mybir.AluOpType


class Dims:
    def __init__(self, B=16, R=29, N=8192, H=768, ncores=8):
        self.B, self.R, self.N, self.H, self.NCORES = B, R, N, H, ncores
        self.NS = N // ncores
        self.ROWS = B * R
        self.KC = H // 128
        self.NBLK = self.NS // 128
        self.NPACK = (R + 7) // 8


DIMS = Dims()


def install_ntff_hook():
    """The image's antenv lacks axon_hooks; inject it so trace=True works."""
    import sys, types
    import antenv

    if "antenv.axon_hooks" in sys.modules:
        return
    mod = types.ModuleType("antenv.axon_hooks")
    mod._hook = None
    mod.set_axon_ntff_profile_hook = lambda h: setattr(mod, "_hook", h)
    mod.get_axon_ntff_profile_hook = lambda: mod._hook
    sys.modules["antenv.axon_hooks"] = mod
    antenv.axon_hooks = mod
    try:
        from trn_agent_boot.trn_boot import _ntff_profile_via_ctypes

        mod.set_axon_ntff_profile_hook(
            _ntff_profile_via_ctypes("/opt/axon/libaxon_pjrt.so")
        )
    except Exception:
        pass


def build_nc(d=DIMS):
    B, R, N, H = d.B, d.R, d.N, d.H
    NCORES, NS, ROWS, KC, NBLK, NPACK = (
        d.NCORES, d.NS, d.ROWS, d.KC, d.NBLK, d.NPACK
    )
    nc = bacc.Bacc("TRN2", target_bir_lowering=False, debug=False, num_devices=NCORES)

    db = nc.dram_tensor("db", [R, NS, H], F32, kind="ExternalInput")
    qT = nc.dram_tensor("qT", [R, H, B], F32, kind="ExternalInput")
    qn = nc.dram_tensor("qn", [NPACK, 128, H], F32, kind="ExternalInput")
    excl = nc.dram_tensor("excl", [NPACK, 128], F32, kind="ExternalInput")
    cbase = nc.dram_tensor("cbase", [128, 1], F32, kind="ExternalInput")
    out = nc.dram_tensor("out", [ROWS, 16], F32, kind="ExternalOutput")

    with TileContext(nc) as tc:
        with (
            tc.tile_pool(name="dram", bufs=1, space="DRAM") as dram,
            tc.tile_pool(name="db_p", bufs=2) as db_p,
            tc.tile_pool(name="dt_p", bufs=2) as dt_p,
            tc.tile_pool(name="psT", bufs=3, space="PSUM") as psT,
            tc.tile_pool(name="psS", bufs=2, space="PSUM") as psS,
            tc.tile_pool(name="small", bufs=2) as small,
            tc.tile_pool(name="persist", bufs=1) as persist,
            tc.tile_pool(name="sims_p", bufs=1) as sims_p,
            tc.tile_pool(name="junk_p", bufs=2) as junk_p,
            tc.tile_pool(name="cand_p", bufs=2) as cand_p,
            tc.tile_pool(name="merge_p", bufs=1) as merge_p,
        ):
            # ---------- one-time setup ----------
            qt_sb = persist.tile([128, R, KC, B], F32, tag="qt")

            iota_f = persist.tile([128, NS], F32, tag="iotaf")
            nc.gpsimd.iota(
                iota_f[:, :],
                pattern=[[1, NS]],
                base=0,
                channel_multiplier=0,
                allow_small_or_imprecise_dtypes=True,
            )
            iota_i = persist.tile([128, 128], I32, tag="iotai")
            nc.gpsimd.iota(
                iota_i[:, :], pattern=[[-1, 128]], base=0, channel_multiplier=1
            )
            eye = persist.tile([128, 128], F32, tag="eye")
            nc.vector.tensor_scalar(
                out=eye[:, :], in0=iota_i[:, :], scalar1=0, scalar2=None,
                op0=ALU.is_equal,
            )

            excl_sb = persist.tile([128, NPACK], F32, tag="excl")
            nc.sync.dma_start(
                out=excl_sb[:, :], in_=excl.ap().rearrange("t p -> p t")
            )
            cbase_sb = persist.tile([128, 1], F32, tag="cbase")
            nc.sync.dma_start(out=cbase_sb[:, :], in_=cbase.ap())

            # query norms in pack layout: rq_pk[16j+b, t] = 1/||q[b, 8t+j]||
            n2q = persist.tile([128, NPACK], F32, tag="n2q")
            rq_pk = persist.tile([128, NPACK], F32, tag="rqpk")
            for tq in range(NPACK):
                qrow = small.tile([128, H], F32, tag="qrow")
                qjunk = junk_p.tile([128, H], F32, tag="qjunk")
                nc.sync.dma_start(out=qrow[:, :], in_=qn[tq, :, :])
                nc.scalar.activation(
                    qjunk[:, :], qrow[:, :], AF.Square,
                    accum_out=n2q[:, tq : tq + 1],
                )
            nc.vector.reciprocal(rq_pk[:, :], n2q[:, :])
            nc.scalar.activation(rq_pk[:, :], rq_pk[:, :], AF.Sqrt)

            # packed sims tiles: pack t holds regions 8t..8t+7 at partitions 16j+b
            sims_pk = [
                sims_p.tile([128, NS], F32, tag=f"simspk{t}", name=f"simspk{t}")
                for t in range(NPACK)
            ]
            for t in range(NPACK):
                nc.vector.memset(sims_pk[t][:, :], NEG)

            cand_t = [
                dram.tile([128, 16], F32, tag=f"cand{t}", name=f"cand{t}")
                for t in range(NPACK)
            ]
            candAG_t = [
                dram.tile([NCORES * 128, 16], F32, tag=f"cAG{t}",
                          name=f"cAG{t}", addr_space="Shared")
                for t in range(NPACK)
            ]

            # ---------- main loop over regions (software-pipelined) ----------
            # Region r's transposes run interleaved (in PE emission order) with
            # region r-1's sims matmuls so the PE never drains and HAM stays warm.

            def emit_tail(rp, dtn_p, rd_b_p):
                # dtn_p is a pair of half tiles [128, KC, NS//2]
                """sims matmul/stage/topk emitters for region rp, as closures."""
                HBw = max(1, NBLK // 2) * 128
                halves = [0, HBw] if NS > HBw else [0]
                hw_ = [HBw, NS - HBw] if NS > HBw else [NS]
                pss_h = [
                    psS.tile([B, hw_[i]], F32, tag="pss", name=f"pss{rp}_{i}")
                    for i in range(len(halves))
                ]
                thunks = []
                for hi, m0 in enumerate(halves):
                    mw = hw_[hi]
                    for k in range(KC):
                        def mm(k=k, hi=hi, mw=mw):
                            nc.tensor.matmul(
                                pss_h[hi][:, :],
                                lhsT=qt_sb[:, rp, k, :],
                                rhs=dtn_p[hi][:, k, 0:mw],
                                start=(k == 0),
                                stop=(k == KC - 1),
                            )
                        thunks.append(mm)

                def finish():
                    # normalize by rd during PSUM->SBUF staging copy.
                    # rd_b free layout is p-major (m = pn*NBLK + a).
                    stage = cand_p.tile([B, NS], F32, tag="stage", name=f"stg{rp}")
                    wv = min(128, NS // 2)
                    apb = wv * NBLK // NS if NS >= 128 else 1
                    for hi, m0 in enumerate(halves):
                        mw = hw_[hi]
                        aw = mw // 128 if mw >= 128 else 1
                        w_ = min(128, mw)
                        nc.vector.tensor_tensor(
                            out=stage[:, m0 : m0 + mw].rearrange(
                                "p (a w) -> p a w", w=w_
                            ),
                            in0=pss_h[hi][:, :].rearrange(
                                "p (a w) -> p a w", w=w_
                            ),
                            in1=rd_b_p[:B, :].rearrange(
                                "p (pn a) -> p a pn", a=NBLK
                            )[:, (m0 // 128) : (m0 // 128) + aw, :]
                            if mw >= 128
                            else rd_b_p[:B, :]
                            .rearrange("p (pn a) -> p pn a", a=NBLK)[
                                :, (m0 % 128) : (m0 % 128) + mw,
                                (m0 // 128) : (m0 // 128) + 1,
                            ]
                            .rearrange("p w o -> p (w o)")
                            .rearrange("p (a w) -> p a w", w=w_),
                            op=ALU.mult,
                        )
                    t, jj = rp // 8, rp % 8
                    nc.sync.dma_start(
                        out=sims_pk[t][16 * jj : 16 * jj + 16, :],
                        in_=stage[:, :],
                    )
                    if jj == 7 or rp == R - 1:
                        nj = jj + 1
                        spk = sims_pk[t]
                        mask = cand_p.tile([128, NS], F32, tag="mask")
                        nc.vector.tensor_scalar(
                            out=mask[:, :],
                            in0=iota_f[:, :],
                            scalar1=excl_sb[:, t : t + 1],
                            scalar2=NEG,
                            op0=ALU.is_equal,
                            op1=ALU.mult,
                        )
                        nc.vector.tensor_tensor(
                            out=spk[:, :], in0=spk[:, :], in1=mask[:, :],
                            op=ALU.add,
                        )
                        max8 = cand_p.tile([128, 8], F32, tag="max8")
                        idx8u = cand_p.tile([128, 8], U32, tag="idx8u")
                        idx8f = cand_p.tile([128, 8], F32, tag="idx8f")
                        nc.vector.max(out=max8[:, :], in_=spk[:, :])
                        nc.vector.max_index(
                            out=idx8u[:, :], in_max=max8[:, :], in_values=spk[:, :]
                        )
                        nc.vector.tensor_scalar(
                            out=idx8f[:, :], in0=idx8u[:, :],
                            scalar1=cbase_sb[:, 0:1], scalar2=None, op0=ALU.add,
                        )
                        nc.sync.dma_start(out=cand_t[t][:, 0:8], in_=max8[:, :])
                        nc.sync.dma_start(out=cand_t[t][:, 8:16], in_=idx8f[:, :])
                        # per-pack all-gather: overlaps with remaining regions
                        nc.gpsimd.collective_compute(
                            "AllGather",
                            ALU.bypass,
                            replica_groups=[list(range(NCORES))],
                            ins=[cand_t[t][:, :].opt()],
                            outs=[candAG_t[t][:, :].opt()],
                        )
                        # per-pack final merge of 64 candidates per row
                        cag = candAG_t[t][:, :].rearrange("(c q) k -> c q k", c=NCORES)
                        fv = merge_p.tile([128, NCORES, 8], F32, tag="fv")
                        fi = merge_p.tile([128, NCORES, 8], F32, tag="fi")
                        nc.sync.dma_start(
                            out=fv[:, :, :],
                            in_=cag[:, :, 0:8].rearrange("c q k -> q c k"),
                        )
                        nc.sync.dma_start(
                            out=fi[:, :, :],
                            in_=cag[:, :, 8:16].rearrange("c q k -> q c k"),
                        )
                        ffv = merge_p.tile([128, 8], F32, tag="ffv")
                        ffi = merge_p.tile([128, 8], F32, tag="ffi")
                        fv2 = fv[:, :, :].rearrange("p c k -> p (c k)")
                        fi2 = fi[:, :, :].rearrange("p c k -> p (c k)")
                        nc.vector.max(out=ffv[:, :], in_=fv2)
                        BIG = 8388608.0  # 2^23 - idx arithmetic exact in fp32
                        fim = merge_p.tile([128, NCORES * 8], F32, tag="fim")
                        nc.vector.tensor_scalar(
                            out=fim[:, :], in0=fi2, scalar1=-BIG, scalar2=None,
                            op0=ALU.add,
                        )
                        for kk in range(8):
                            fmask = merge_p.tile([128, NCORES * 8], F32, tag="fmask")
                            nc.vector.tensor_scalar(
                                out=fmask[:, :], in0=fv2,
                                scalar1=ffv[:, kk : kk + 1],
                                scalar2=None, op0=ALU.is_equal,
                            )
                            nc.vector.tensor_tensor(
                                out=fmask[:, :], in0=fmask[:, :], in1=fim[:, :],
                                op=ALU.mult,
                            )
                            nc.vector.tensor_reduce(
                                out=ffi[:, kk : kk + 1], in_=fmask[:, :],
                                axis=mybir.AxisListType.X, op=ALU.min,
                            )
                        nc.vector.tensor_scalar(
                            out=ffi[:, :], in0=ffi[:, :], scalar1=BIG,
                            scalar2=None, op0=ALU.add,
                        )
                        nc.vector.tensor_scalar(
                            out=ffv[:, :], in0=ffv[:, :],
                            scalar1=rq_pk[:, t : t + 1], scalar2=None,
                            op0=ALU.mult,
                        )
                        # out rows b*R + (8t + j) from pack partitions 16j+b
                        ov = out[:, :].rearrange("(b r) k -> b r k", b=B)
                        for j2 in range(nj):
                            nc.sync.dma_start(
                                out=ov[:, 8 * t + j2, 0:8],
                                in_=ffv[16 * j2 : 16 * j2 + 16, :],
                            )
                            nc.sync.dma_start(
                                out=ov[:, 8 * t + j2, 8:16],
                                in_=ffi[16 * j2 : 16 * j2 + 16, :],
                            )

                thunks.append(finish)
                return thunks

            pending = []
            for r in range(R):
                d_sb = db_p.tile([128, NBLK, H], F32, tag="d")
                nc.sync.dma_start(
                    out=d_sb[:, :, :],
                    in_=db[r, :, :].rearrange("(a p) h -> p a h", p=128),
                )
                if r == 0:
                    # qt load deferred so the first db tile DMA goes out first
                    nc.sync.dma_start(
                        out=qt_sb[:, :, :, :],
                        in_=qT.ap().rearrange("r (k p) b -> p r k b", p=128),
                    )
                # db row norms^2 per n-block (split ACT / GPS+DVE / DVE-stt)
                n2d = small.tile([128, NBLK], F32, tag="n2d")
                for a in range(NBLK):
                    if a % 8 < NORM_ACT:
                        djunk = junk_p.tile([128, H], F32, tag="djunk")
                        nc.scalar.activation(
                            djunk[:, :], d_sb[:, a, :], AF.Square,
                            accum_out=n2d[:, a : a + 1],
                        )
                    elif a % 8 < NORM_ACT + NORM_GPS:
                        gjunk = junk_p.tile([128, H], F32, tag="gjunk")
                        nc.gpsimd.tensor_tensor(
                            out=gjunk[:, :], in0=d_sb[:, a, :],
                            in1=d_sb[:, a, :], op=ALU.mult,
                        )
                        nc.vector.tensor_reduce(
                            out=n2d[:, a : a + 1], in_=gjunk[:, :],
                            axis=mybir.AxisListType.X, op=ALU.add,
                        )
                    else:
                        vjunk = junk_p.tile([128, H], F32, tag="vjunk")
                        nc.vector.scalar_tensor_tensor(
                            out=vjunk[:, :], in0=d_sb[:, a, :], scalar=1.0,
                            in1=d_sb[:, a, :], op0=ALU.mult, op1=ALU.mult,
                            accum_out=n2d[:, a : a + 1],
                        )
                rd = small.tile([128, NBLK], F32, tag="rd")
                nc.vector.reciprocal(rd[:, :], n2d[:, :])
                nc.scalar.activation(rd[:, :], rd[:, :], AF.Sqrt)

                # rd broadcast tile (16 partitions): flatten-DMA rd to one
                # partition (p-major), then one partition_broadcast per region.
                rdt_flat = small.tile([1, NS], F32, tag="rdtflat")
                nc.sync.dma_start(out=rdt_flat[:, :], in_=rd[:, :])
                rd_b = small.tile([B, NS], F32, tag="rdb")
                nc.gpsimd.partition_broadcast(rd_b[:, :], rdt_flat[:, :])

                # transposes for region r interleaved with region r-1's sims MMs
                # dtn split into n-halves so sims MMs unblock after half the copies
                HB = max(1, NBLK // 2)  # n-blocks per half
                dtn = [
                    dt_p.tile([128, KC, HB * 128], F32, tag="dtnA", name=f"dtnA{r}"),
                    dt_p.tile([128, KC, NS - HB * 128], F32, tag="dtnB",
                              name=f"dtnB{r}")
                    if NS - HB * 128 > 0
                    else None,
                ]
                for a in range(NBLK):
                    pst = psT.tile([128, H], F32, tag="pst")
                    for k in range(KC):
                        nc.tensor.matmul(
                            pst[:, k * 128 : (k + 1) * 128],
                            lhsT=d_sb[:, a, k * 128 : (k + 1) * 128],
                            rhs=eye[:, :],
                            start=True, stop=True,
                            is_transpose=True,
                        )
                    hi2, ao = (0, a) if a < HB else (1, a - HB)
                    dst = dtn[hi2][:, :, ao * 128 : (ao + 1) * 128]
                    src = pst[:, :].rearrange("p (k h) -> p k h", k=KC)
                    if a % 8 < COPY_ACT:
                        nc.scalar.activation(dst, src, AF.Copy)
                    else:
                        nc.vector.tensor_copy(dst, src)
                    # interleave ~2 pending sims-MMs of the previous region
                    for _ in range(2):
                        if pending:
                            pending.pop(0)()
                while pending:
                    pending.pop(0)()
                pending = emit_tail(r, dtn, rd_b)
            while pending:
                pending.pop(0)()

    nc.finalize()
    return nc


_NC_CACHE = {}


def _get_nc():
    if "nc" not in _NC_CACHE:
        _NC_CACHE["nc"] = build_nc()
    return _NC_CACHE["nc"]


def make_in_maps(query, db_features, exclude_idx, d=DIMS):
    B, R, NCORES, NS, NPACK = d.B, d.R, d.NCORES, d.NS, d.NPACK
    query = np.asarray(query, dtype=np.float32)
    db_features = np.asarray(db_features, dtype=np.float32)
    exclude_idx = np.asarray(exclude_idx, dtype=np.int32)

    qT = np.ascontiguousarray(query.transpose(1, 2, 0))  # [R, H, B]
    # qn pack layout: qn[t, 16j+b, :] = query[b, 8t+j, :], gaps = ones
    H_ = query.shape[2]
    qn = np.ones((NPACK, 128, H_), dtype=np.float32)
    for r in range(R):
        t, j = r // 8, r % 8
        qn[t, 16 * j : 16 * j + B, :] = query[:, r, :]

    in_maps = []
    for c in range(NCORES):
        db_c = np.ascontiguousarray(db_features[:, c * NS : (c + 1) * NS, :])
        ex = np.full((NPACK, 128), -1.0, dtype=np.float32)
        for r in range(R):
            t, j = r // 8, r % 8
            for b in range(B):
                e = int(exclude_idx[b, r]) - c * NS
                if 0 <= e < NS:
                    ex[t, 16 * j + b] = float(e)
        cb = np.full((128, 1), float(c * NS), dtype=np.float32)
        in_maps.append(
            {"db": db_c, "qT": qT, "qn": qn, "excl": ex, "cbase": cb}
        )
    return in_maps


def run_device(query, db_features, exclude_idx, trace=False):
    install_ntff_hook()
    nc = _get_nc()
    in_maps = make_in_maps(query, db_features, exclude_idx)
    res = run_bass_kernel_spmd(
        nc, in_maps, core_ids=list(range(NCORES)), trace=trace
    )
    return res


def kernel(query, db_features, exclude_idx, top_k):
    k = int(np.asarray(top_k))
    assert k == 8, f"kernel hardcodes top_k=8, got {k}"
    res = run_device(query, db_features, exclude_idx, trace=False)
    o = res.results[0]["out"]  # [464, 16]
    top_vals = np.ascontiguousarray(o[:, :8].reshape(B, R, 8).astype(np.float32))
    top_idx = np.ascontiguousarray(o[:, 8:16].reshape(B, R, 8).astype(np.int32))
    best = np.ascontiguousarray(top_vals[..., 0])
    return top_vals, top_idx, best


# revision 42
# speedup vs baseline: 1.0957x; 1.0957x over previous
"""Sharded KNN retrieval kernel for trn2 (8 NeuronCores).

Problem: nn_AnatomicalTextEnhancer — cosine-similarity top-k per (batch, region)
  query       [16, 29, 768] f32
  db_features [29, 8192, 768] f32
  exclude_idx [16, 29] i32
  top_k = 8
Returns (top_vals [16,29,8] f32, top_idx [16,29,8] i32, best_similarity [16,29] f32).

Sharding: db along N (1024 samples/core), query replicated; per-core local
top-8 then AllGather + re-top-8 of 64 on device (standard sharded KNN).

Per-core pipeline (fp32 exact; ~810-890 us HW exec):
  - db region tiles DMA'd natural [n, h] (3 MB strided loads, full HBM BW)
  - row norms^2 via Square+accum, split across ScalarE/GpSimd+DVE (knobs)
  - db transposed to [h, n] via PE transpose-mode matmuls (1-pass fp32),
    PSUM->SBUF copies split ScalarE/DVE
  - sims = qT.T @ dbT fp32 matmuls (M=16, N=512, PSUM-accumulated over 6
    h-chunks), software-pipelined one region behind the transposes so the
    PE stays dense
  - 1/||d|| folded into the PSUM->SBUF staging copy (free-axis multiply
    against a partition_broadcast row); 1/||q|| applied only to the 8
    final values (row-scaling cannot change ranking)
  - exclusion pre-masked via iota==local_excl -> -1e30 on packed
    [128, 1024] tiles (8 regions x 16 batch rows per pack)
  - local top-8 via DVE max/max_index; candidates (val, global idx as f32)
    AllGathered; final top-8-of-64 via max + exact value-match index
    recovery (min over idx-2^23 restricted to matches, ties -> lowest idx)
"""

import numpy as np

import concourse.bass as bass
import concourse.bacc as bacc
import concourse.mybir as mybir
from concourse.tile import TileContext
from concourse.bass_utils import run_bass_kernel_spmd

F32 = mybir.dt.float32
I32 = mybir.dt.int32
U16 = mybir.dt.uint16
U32 = mybir.dt.uint32

B, R, N, H = 16, 29, 8192, 768
NCORES = 8
NS = N // NCORES          # 1024 db samples per core
ROWS = B * R              # 464 (b-major: row = b*29 + r)
KC = H // 128             # 6 h-chunks
NBLK = NS // 128          # 8 n-blocks per region per core
NPACK = (R + 7) // 8      # 4 packs of 8 regions for 128-partition top-k tiles
NEG = -1e30
NORM_ACT = 4   # n-blocks per 8 whose norms run on ACT
NORM_GPS = 3   # n-blocks per 8 whose norm squares run on GPS (DVE reduces)
               # remaining blocks: DVE 1-pass via scalar_tensor_tensor accum
COPY_ACT = 5   # n-blocks per 8 whose PSUM->SBUF DT copy runs on ACT (rest DVE)
AF = mybir.ActivationFunctionType
ALU = mybir.AluOpType


class Dims:
    def __init__(self, B=16, R=29, N=8192, H=768, ncores=8):
        self.B, self.R, self.N, self.H, self.NCORES = B, R, N, H, ncores
        self.NS = N // ncores
        self.ROWS = B * R
        self.KC = H // 128
        self.NBLK = self.NS // 128
        self.NPACK = (R + 7) // 8


DIMS = Dims()


def install_ntff_hook():
    """The image's antenv lacks axon_hooks; inject it so trace=True works."""
    import sys, types
    import antenv

    if "antenv.axon_hooks" in sys.modules:
        return
    mod = types.ModuleType("antenv.axon_hooks")
    mod._hook = None
    mod.set_axon_ntff_profile_hook = lambda h: setattr(mod, "_hook", h)
    mod.get_axon_ntff_profile_hook = lambda: mod._hook
    sys.modules["antenv.axon_hooks"] = mod
    antenv.axon_hooks = mod
    try:
        from trn_agent_boot.trn_boot import _ntff_profile_via_ctypes

        mod.set_axon_ntff_profile_hook(
            _ntff_profile_via_ctypes("/opt/axon/libaxon_pjrt.so")
        )
    except Exception:
        pass


def build_nc(d=DIMS):
    B, R, N, H = d.B, d.R, d.N, d.H
    NCORES, NS, ROWS, KC, NBLK, NPACK = (
        d.NCORES, d.NS, d.ROWS, d.KC, d.NBLK, d.NPACK
    )
    nc = bacc.Bacc("TRN2", target_bir_lowering=False, debug=False, num_devices=NCORES)

    db = nc.dram_tensor("db", [R, NS, H], F32, kind="ExternalInput")
    qT = nc.dram_tensor("qT", [R, H, B], F32, kind="ExternalInput")
    qn = nc.dram_tensor("qn", [NPACK, 128, H], F32, kind="ExternalInput")
    excl = nc.dram_tensor("excl", [NPACK, 128], F32, kind="ExternalInput")
    cbase = nc.dram_tensor("cbase", [128, 1], F32, kind="ExternalInput")
    out = nc.dram_tensor("out", [ROWS, 16], F32, kind="ExternalOutput")

    with TileContext(nc) as tc:
        with (
            tc.tile_pool(name="dram", bufs=1, space="DRAM") as dram,
            tc.tile_pool(name="db_p", bufs=2) as db_p,
            tc.tile_pool(name="dt_p", bufs=2) as dt_p,
            tc.tile_pool(name="psT", bufs=3, space="PSUM") as psT,
            tc.tile_pool(name="psS", bufs=2, space="PSUM") as psS,
            tc.tile_pool(name="small", bufs=2) as small,
            tc.tile_pool(name="persist", bufs=1) as persist,
            tc.tile_pool(name="sims_p", bufs=1) as sims_p,
            tc.tile_pool(name="junk_p", bufs=2) as junk_p,
            tc.tile_pool(name="cand_p", bufs=2) as cand_p,
            tc.tile_pool(name="merge_p", bufs=1) as merge_p,
        ):
            # ---------- one-time setup ----------
            qt_sb = persist.tile([128, R, KC, B], F32, tag="qt")

            iota_f = persist.tile([128, NS], F32, tag="iotaf")
            nc.gpsimd.iota(
                iota_f[:, :],
                pattern=[[1, NS]],
                base=0,
                channel_multiplier=0,
                allow_small_or_imprecise_dtypes=True,
            )
            iota_i = persist.tile([128, 128], I32, tag="iotai")
            nc.gpsimd.iota(
                iota_i[:, :], pattern=[[-1, 128]], base=0, channel_multiplier=1
            )
            eye = persist.tile([128, 128], F32, tag="eye")
            nc.vector.tensor_scalar(
                out=eye[:, :], in0=iota_i[:, :], scalar1=0, scalar2=None,
                op0=ALU.is_equal,
            )

            excl_sb = persist.tile([128, NPACK], F32, tag="excl")
            nc.sync.dma_start(
                out=excl_sb[:, :], in_=excl.ap().rearrange("t p -> p t")
            )
            cbase_sb = persist.tile([128, 1], F32, tag="cbase")
            nc.sync.dma_start(out=cbase_sb[:, :], in_=cbase.ap())

            # query norms in pack layout: rq_pk[16j+b, t] = 1/||q[b, 8t+j]||
            n2q = persist.tile([128, NPACK], F32, tag="n2q")
            rq_pk = persist.tile([128, NPACK], F32, tag="rqpk")
            for tq in range(NPACK):
                qrow = small.tile([128, H], F32, tag="qrow")
                qjunk = junk_p.tile([128, H], F32, tag="qjunk")
                nc.sync.dma_start(out=qrow[:, :], in_=qn[tq, :, :])
                nc.scalar.activation(
                    qjunk[:, :], qrow[:, :], AF.Square,
                    accum_out=n2q[:, tq : tq + 1],
                )
            nc.vector.reciprocal(rq_pk[:, :], n2q[:, :])
            nc.scalar.activation(rq_pk[:, :], rq_pk[:, :], AF.Sqrt)

            # packed sims tiles: pack t holds regions 8t..8t+7 at partitions 16j+b
            sims_pk = [
                sims_p.tile([128, NS], F32, tag=f"simspk{t}", name=f"simspk{t}")
                for t in range(NPACK)
            ]
            for t in range(NPACK):
                nc.vector.memset(sims_pk[t][:, :], NEG)

            cand_t = [
                dram.tile([128, 16], F32, tag=f"cand{t}", name=f"cand{t}")
                for t in range(NPACK)
            ]
            candAG_t = [
                dram.tile([NCORES * 128, 16], F32, tag=f"cAG{t}",
                          name=f"cAG{t}", addr_space="Shared")
                for t in range(NPACK)
            ]

            # ---------- main loop over regions (software-pipelined) ----------
            # Region r's transposes run interleaved (in PE emission order) with
            # region r-1's sims matmuls so the PE never drains and HAM stays warm.

            def emit_tail(rp, dtn_p, rd_b_p):
                """sims matmul/stage/topk emitters for region rp, as closures."""
                halves = list(range(0, NS, 512))
                hw_ = [min(512, NS - m0) for m0 in halves]
                pss_h = [
                    psS.tile([B, hw_[i]], F32, tag="pss", name=f"pss{rp}_{i}")
                    for i in range(len(halves))
                ]
                thunks = []
                for hi, m0 in enumerate(halves):
                    mw = hw_[hi]
                    for k in range(KC):
                        def mm(k=k, hi=hi, mw=mw):
                            nc.tensor.matmul(
                                pss_h[hi][:, :],
                                lhsT=qt_sb[:, rp, k, :],
                                rhs=dtn_p[:, k, halves[hi] : halves[hi] + mw],
                                start=(k == 0),
                                stop=(k == KC - 1),
                            )
                        thunks.append(mm)

                def finish():
                    # normalize by rd during PSUM->SBUF staging copy.
                    # rd_b free layout is p-major (m = pn*NBLK + a).
                    stage = cand_p.tile([B, NS], F32, tag="stage", name=f"stg{rp}")
                    wv = min(128, NS // 2)
                    apb = wv * NBLK // NS if NS >= 128 else 1
                    for hi, m0 in enumerate(halves):
                        mw = hw_[hi]
                        aw = mw // 128 if mw >= 128 else 1
                        w_ = min(128, mw)
                        nc.vector.tensor_tensor(
                            out=stage[:, m0 : m0 + mw].rearrange(
                                "p (a w) -> p a w", w=w_
                            ),
                            in0=pss_h[hi][:, :].rearrange(
                                "p (a w) -> p a w", w=w_
                            ),
                            in1=rd_b_p[:B, :].rearrange(
                                "p (pn a) -> p a pn", a=NBLK
                            )[:, (m0 // 128) : (m0 // 128) + aw, :]
                            if mw >= 128
                            else rd_b_p[:B, :]
                            .rearrange("p (pn a) -> p pn a", a=NBLK)[
                                :, (m0 % 128) : (m0 % 128) + mw,
                                (m0 // 128) : (m0 // 128) + 1,
                            ]
                            .rearrange("p w o -> p (w o)")
                            .rearrange("p (a w) -> p a w", w=w_),
                            op=ALU.mult,
                        )
                    t, jj = rp // 8, rp % 8
                    nc.sync.dma_start(
                        out=sims_pk[t][16 * jj : 16 * jj + 16, :],
                        in_=stage[:, :],
                    )
                    if jj == 7 or rp == R - 1:
                        nj = jj + 1
                        spk = sims_pk[t]
                        mask = cand_p.tile([128, NS], F32, tag="mask")
                        nc.vector.tensor_scalar(
                            out=mask[:, :],
                            in0=iota_f[:, :],
                            scalar1=excl_sb[:, t : t + 1],
                            scalar2=NEG,
                            op0=ALU.is_equal,
                            op1=ALU.mult,
                        )
                        nc.vector.tensor_tensor(
                            out=spk[:, :], in0=spk[:, :], in1=mask[:, :],
                            op=ALU.add,
                        )
                        max8 = cand_p.tile([128, 8], F32, tag="max8")
                        idx8u = cand_p.tile([128, 8], U32, tag="idx8u")
                        idx8f = cand_p.tile([128, 8], F32, tag="idx8f")
                        nc.vector.max(out=max8[:, :], in_=spk[:, :])
                        nc.vector.max_index(
                            out=idx8u[:, :], in_max=max8[:, :], in_values=spk[:, :]
                        )
                        nc.vector.tensor_scalar(
                            out=idx8f[:, :], in0=idx8u[:, :],
                            scalar1=cbase_sb[:, 0:1], scalar2=None, op0=ALU.add,
                        )
                        nc.sync.dma_start(out=cand_t[t][:, 0:8], in_=max8[:, :])
                        nc.sync.dma_start(out=cand_t[t][:, 8:16], in_=idx8f[:, :])
                        # per-pack all-gather: overlaps with remaining regions
                        nc.gpsimd.collective_compute(
                            "AllGather",
                            ALU.bypass,
                            replica_groups=[list(range(NCORES))],
                            ins=[cand_t[t][:, :].opt()],
                            outs=[candAG_t[t][:, :].opt()],
                        )
                        # per-pack final merge of 64 candidates per row
                        cag = candAG_t[t][:, :].rearrange("(c q) k -> c q k", c=NCORES)
                        fv = merge_p.tile([128, NCORES, 8], F32, tag="fv")
                        fi = merge_p.tile([128, NCORES, 8], F32, tag="fi")
                        nc.sync.dma_start(
                            out=fv[:, :, :],
                            in_=cag[:, :, 0:8].rearrange("c q k -> q c k"),
                        )
                        nc.sync.dma_start(
                            out=fi[:, :, :],
                            in_=cag[:, :, 8:16].rearrange("c q k -> q c k"),
                        )
                        ffv = merge_p.tile([128, 8], F32, tag="ffv")
                        ffi = merge_p.tile([128, 8], F32, tag="ffi")
                        fv2 = fv[:, :, :].rearrange("p c k -> p (c k)")
                        fi2 = fi[:, :, :].rearrange("p c k -> p (c k)")
                        nc.vector.max(out=ffv[:, :], in_=fv2)
                        BIG = 8388608.0  # 2^23 - idx arithmetic exact in fp32
                        fim = merge_p.tile([128, NCORES * 8], F32, tag="fim")
                        nc.vector.tensor_scalar(
                            out=fim[:, :], in0=fi2, scalar1=-BIG, scalar2=None,
                            op0=ALU.add,
                        )
                        for kk in range(8):
                            fmask = merge_p.tile([128, NCORES * 8], F32, tag="fmask")
                            nc.vector.tensor_scalar(
                                out=fmask[:, :], in0=fv2,
                                scalar1=ffv[:, kk : kk + 1],
                                scalar2=None, op0=ALU.is_equal,
                            )
                            nc.vector.tensor_tensor(
                                out=fmask[:, :], in0=fmask[:, :], in1=fim[:, :],
                                op=ALU.mult,
                            )
                            nc.vector.tensor_reduce(
                                out=ffi[:, kk : kk + 1], in_=fmask[:, :],
                                axis=mybir.AxisListType.X, op=ALU.min,
                            )
                        nc.vector.tensor_scalar(
                            out=ffi[:, :], in0=ffi[:, :], scalar1=BIG,
                            scalar2=None, op0=ALU.add,
                        )
                        nc.vector.tensor_scalar(
                            out=ffv[:, :], in0=ffv[:, :],
                            scalar1=rq_pk[:, t : t + 1], scalar2=None,
                            op0=ALU.mult,
                        )
                        # out rows b*R + (8t + j) from pack partitions 16j+b
                        ov = out[:, :].rearrange("(b r) k -> b r k", b=B)
                        for j2 in range(nj):
                            nc.sync.dma_start(
                                out=ov[:, 8 * t + j2, 0:8],
                                in_=ffv[16 * j2 : 16 * j2 + 16, :],
                            )
                            nc.sync.dma_start(
                                out=ov[:, 8 * t + j2, 8:16],
                                in_=ffi[16 * j2 : 16 * j2 + 16, :],
                            )

                thunks.append(finish)
                return thunks

            pending = []
            for r in range(R):
                d_sb = db_p.tile([128, NBLK, H], F32, tag="d")
                nc.sync.dma_start(
                    out=d_sb[:, :, :],
                    in_=db[r, :, :].rearrange("(a p) h -> p a h", p=128),
                )
                if r == 0:
                    # qt load deferred so the first db tile DMA goes out first
                    nc.sync.dma_start(
                        out=qt_sb[:, :, :, :],
                        in_=qT.ap().rearrange("r (k p) b -> p r k b", p=128),
                    )
                # db row norms^2 per n-block (split ACT / GPS+DVE / DVE-stt)
                n2d = small.tile([128, NBLK], F32, tag="n2d")
                for a in range(NBLK):
                    if a % 8 < NORM_ACT:
                        djunk = junk_p.tile([128, H], F32, tag="djunk")
                        nc.scalar.activation(
                            djunk[:, :], d_sb[:, a, :], AF.Square,
                            accum_out=n2d[:, a : a + 1],
                        )
                    elif a % 8 < NORM_ACT + NORM_GPS:
                        gjunk = junk_p.tile([128, H], F32, tag="gjunk")
                        nc.gpsimd.tensor_tensor(
                            out=gjunk[:, :], in0=d_sb[:, a, :],
                            in1=d_sb[:, a, :], op=ALU.mult,
                        )
                        nc.vector.tensor_reduce(
                            out=n2d[:, a : a + 1], in_=gjunk[:, :],
                            axis=mybir.AxisListType.X, op=ALU.add,
                        )
                    else:
                        vjunk = junk_p.tile([128, H], F32, tag="vjunk")
                        nc.vector.scalar_tensor_tensor(
                            out=vjunk[:, :], in0=d_sb[:, a, :], scalar=1.0,
                            in1=d_sb[:, a, :], op0=ALU.mult, op1=ALU.mult,
                            accum_out=n2d[:, a : a + 1],
                        )
                rd = small.tile([128, NBLK], F32, tag="rd")
                nc.vector.reciprocal(rd[:, :], n2d[:, :])
                nc.scalar.activation(rd[:, :], rd[:, :], AF.Sqrt)

                # rd broadcast tile (16 partitions): flatten-DMA rd to one
                # partition (p-major), then one partition_broadcast per region.
                rdt_flat = small.tile([1, NS], F32, tag="rdtflat")
                nc.sync.dma_start(out=rdt_flat[:, :], in_=rd[:, :])
                rd_b = small.tile([B, NS], F32, tag="rdb")
                nc.gpsimd.partition_broadcast(rd_b[:, :], rdt_flat[:, :])

                # transposes for region r interleaved with region r-1's sims MMs
                dtn = dt_p.tile([128, KC, NS], F32, tag="dtn")
                for a in range(NBLK):
                    pst = psT.tile([128, H], F32, tag="pst")
                    for k in range(KC):
                        nc.tensor.matmul(
                            pst[:, k * 128 : (k + 1) * 128],
                            lhsT=d_sb[:, a, k * 128 : (k + 1) * 128],
                            rhs=eye[:, :],
                            start=True, stop=True,
                            is_transpose=True,
                        )
                    dst = dtn[:, :, a * 128 : (a + 1) * 128]
                    src = pst[:, :].rearrange("p (k h) -> p k h", k=KC)
                    if a % 8 < COPY_ACT:
                        nc.scalar.activation(dst, src, AF.Copy)
                    else:
                        nc.vector.tensor_copy(dst, src)
                    # interleave ~2 pending sims-MMs of the previous region
                    for _ in range(2):
                        if pending:
                            pending.pop(0)()
                while pending:
                    pending.pop(0)()
                pending = emit_tail(r, dtn, rd_b)
            while pending:
                pending.pop(0)()

    nc.finalize()
    return nc


_NC_CACHE = {}


def _get_nc():
    if "nc" not in _NC_CACHE:
        _NC_CACHE["nc"] = build_nc()
    return _NC_CACHE["nc"]


def make_in_maps(query, db_features, exclude_idx, d=DIMS):
    B, R, NCORES, NS, NPACK = d.B, d.R, d.NCORES, d.NS, d.NPACK
    query = np.asarray(query, dtype=np.float32)
    db_features = np.asarray(db_features, dtype=np.float32)
    exclude_idx = np.asarray(exclude_idx, dtype=np.int32)

    qT = np.ascontiguousarray(query.transpose(1, 2, 0))  # [R, H, B]
    # qn pack layout: qn[t, 16j+b, :] = query[b, 8t+j, :], gaps = ones
    H_ = query.shape[2]
    qn = np.ones((NPACK, 128, H_), dtype=np.float32)
    for r in range(R):
        t, j = r // 8, r % 8
        qn[t, 16 * j : 16 * j + B, :] = query[:, r, :]

    in_maps = []
    for c in range(NCORES):
        db_c = np.ascontiguousarray(db_features[:, c * NS : (c + 1) * NS, :])
        ex = np.full((NPACK, 128), -1.0, dtype=np.float32)
        for r in range(R):
            t, j = r // 8, r % 8
            for b in range(B):
                e = int(exclude_idx[b, r]) - c * NS
                if 0 <= e < NS:
                    ex[t, 16 * j + b] = float(e)
        cb = np.full((128, 1), float(c * NS), dtype=np.float32)
        in_maps.append(
            {"db": db_c, "qT": qT, "qn": qn, "excl": ex, "cbase": cb}
        )
    return in_maps


def run_device(query, db_features, exclude_idx, trace=False):
    install_ntff_hook()
    nc = _get_nc()
    in_maps = make_in_maps(query, db_features, exclude_idx)
    res = run_bass_kernel_spmd(
        nc, in_maps, core_ids=list(range(NCORES)), trace=trace
    )
    return res


def kernel(query, db_features, exclude_idx, top_k):
    k = int(np.asarray(top_k))
    assert k == 8, f"kernel hardcodes top_k=8, got {k}"
    res = run_device(query, db_features, exclude_idx, trace=False)
    o = res.results[0]["out"]  # [464, 16]
    top_vals = np.ascontiguousarray(o[:, :8].reshape(B, R, 8).astype(np.float32))
    top_idx = np.ascontiguousarray(o[:, 8:16].reshape(B, R, 8).astype(np.int32))
    best = np.ascontiguousarray(top_vals[..., 0])
    return top_vals, top_idx, best
